# revision 17
# baseline (speedup 1.0000x reference)
"""Trainium2 Bass kernel for nn_DotAttention (B=8, JX=JM=2048, D=H=512).

Sharding: data-parallel over batch B — one batch element per NeuronCore
(8 cores). Weights replicated. Per example:

    q  = relu(x @ Wq)          k = relu(mem @ Wk)
    s  = q @ k^T / sqrt(H)     p = exp(s + (mask-1)*1e30 - C)   (C=5: scores
                               are bounded ~[1.9, 8.8], so exp(s-C) <= ~50
                               fits fp8e4m3 and no row-max pass is needed)
    att = (p @ mem) / colsum(p)
    res = [x, att];  out = res * sigmoid(res @ Wg)

MASK COMPACTION: masked memory slots contribute EXACTLY zero (exp of
-1e30) to att and L, so the host gathers only the valid rows of
`memory` (per example, ~50% of JM) and pads to a multiple of 256.
kproj/scores/colsum/att then run on JM_pad ~ 1280 instead of 2048 —
identical math, ~37% less work in the attention path.

Precision plan (tolerance 2e-2 scale-relative; fp8 peak is 157 TF/s =
2x bf16, i.e. a 256-contract DoubleRow instruction costs the same
cycles/column as a 128-contract bf16 one):
  fp8e4m3 DoubleRow for kproj/qproj/scores/att and the gate GEMM's
  att-half; fp16 for the gate GEMM's x-half (x quantization error
  dominates the gate logits at fp8); fp16 output DMA (host upcasts).

The gate's sigmoid is computed as tanh: sigmoid(z) = 0.5*(1+tanh(z/2)),
because Tanh lives in the SAME activation-function table as Exp while
Sigmoid does not — interleaving exp and sigmoid on ACT would cost a
1283ns table reload per switch. The device ships OUT2 = (1 + tanh(z/2)) * res = 2*out
(fused into one DVE scalar_tensor_tensor) and the HOST applies the 0.5
while upcasting. x itself is reconstructed on-device from two fp8
halves (x8t + x8rt), saving 1MB of HBM per core per iteration — at 8
cores the chip is HBM-bandwidth-bound (~95GB/s/core effective), so
every MB costs ~10.5us.

Schedule: PE work is interleaved so the ACT exp drain (which PSUM 's'
buffers rotate 3-deep against) always has independent matmuls to
overlap with:

  R0: kproj(n=0) + qproj(b=0) interleaved (k-relus on ACT, q on DVE)
  R3: scores0 all tiles, interleaved with kproj(n=1) | qproj(b=1) |
        early colsum0 steps
  R5: att0 interleaved with the first scores1 tiles
  R6: gate0 interleaved with the last scores1 tiles | colsum1
  R8: att1, gate1

DMA: ALL input triggers ride the Pool queue in prev-iteration
WAR-release order (Pool's sequencer wraps earliest, so inputs prefetch
during the previous iteration); outputs ride the SP queue.
All transposed operands are prepared on the HOST.
"""

import sys

for _p in ("/opt/trn_rl_repo",):
    if _p not in sys.path:
        sys.path.insert(0, _p)

import numpy as np

import concourse.bass as bass
import concourse.mybir as mybir
import concourse.tile as tile
from concourse import bacc
from concourse.bass_utils import run_bass_kernel_spmd
from contextlib import ExitStack

F32 = mybir.dt.float32
F16 = mybir.dt.float16
F8 = mybir.dt.float8e4

P = 128
JX = 2048
JM = 2048
D = 512
H = 512
E = 2 * D
N_CORES = 8
SCALE = 1.0 / float(np.sqrt(H))
CEXP = 5.0          # exp offset folded into the mask bias
WSCALE = 32.0       # gate weights are quantized at 32x; tanh rescales
BLK = 1024

Act = mybir.ActivationFunctionType
Alu = mybir.AluOpType
DR = mybir.MatmulPerfMode.DoubleRow

DC = D // P    # 4
HC = H // P    # 4
EC = E // P    # 8
NBLK = JX // BLK

# set by _make_in_maps (compacted memory length); 2048 = no compaction
_LAST_JM_PAD = [JM]


def build_program_v2(hw_loop=None, iters=1, enable_asserts=False, jm_pad=None):
    if jm_pad is None:
        jm_pad = _LAST_JM_PAD[0]
    MC = jm_pad // P
    nc = bacc.Bacc("TRN2", target_bir_lowering=False, debug=False,
                   enable_asserts=enable_asserts)

    x8t_d = nc.dram_tensor("x8t", [D, JX], F8, kind="ExternalInput")
    x8rt_d = nc.dram_tensor("x8rt", [D, JX], F8, kind="ExternalInput")
    m8_d = nc.dram_tensor("m8", [jm_pad, D], F8, kind="ExternalInput")
    m8t_d = nc.dram_tensor("m8t", [D, jm_pad], F8, kind="ExternalInput")
    addm_d = nc.dram_tensor("addm", [P, MC], F32, kind="ExternalInput")
    wq8_d = nc.dram_tensor("wq8", [D, H], F8, kind="ExternalInput")
    wk8_d = nc.dram_tensor("wk8", [D, H], F8, kind="ExternalInput")
    wg8a_d = nc.dram_tensor("wg8a", [D, E], F8, kind="ExternalInput")
    wg8b_d = nc.dram_tensor("wg8b", [D, E], F8, kind="ExternalInput")
    wga8_d = nc.dram_tensor("wga8", [D, E], F8, kind="ExternalInput")
    out_d = nc.dram_tensor("out", [E, JX], F16, kind="ExternalOutput")

    def mm(ps, lhsT, rhs, start, stop):
        nc.tensor.matmul(ps, lhsT, rhs, start=start, stop=stop)

    def mm8(ps, lhsT, rhs, start, stop):
        nc.tensor.matmul(ps, lhsT, rhs, start=start, stop=stop, perf_mode=DR)

    with tile.TileContext(nc) as tc, \
         nc.allow_low_precision(reason="fp8/fp16 mixed-precision plan, "
                                "validated vs 2e-2 tolerance"):
      with ExitStack() as ctx:
        const = ctx.enter_context(tc.tile_pool(name="const", bufs=1))
        ones_f = const.tile([P, 2, P], F32)
        nc.vector.memset(ones_f[:], 1.0)
        ones_8 = const.tile([P, 2, P], F8)
        nc.scalar.copy(ones_8[:], ones_f[:])

        persist = ctx.enter_context(tc.tile_pool(name="persist", bufs=1))
        arena = ctx.enter_context(tc.tile_pool(name="arena", bufs=1))
        small = ctx.enter_context(tc.tile_pool(name="small", bufs=2))
        psb = ctx.enter_context(tc.tile_pool(name="psb", bufs=1, space="PSUM"))

        def body(_iv=None):
            # ---- input DMA triggers, ALL on the Pool queue, ordered by the
            # previous iteration's WAR-release time of each destination tile
            # (a blocked trigger stalls Pool.SEQ and every later trigger).
            m8t_sb = arena.tile([P, DC, jm_pad], F8, tag="m8t", name="m8t_sb", bufs=2)
            m8t_r = m8t_d.ap().rearrange("(c p) j -> p c j", p=P)
            half = min(1024, jm_pad)
            nc.gpsimd.dma_start(out=m8t_sb[:, :, 0:half], in_=m8t_r[:, :, 0:half])
            wk8_sb = small.tile([P, DC, H], F8, tag="wk8", name="wk8_sb", bufs=2)
            nc.gpsimd.dma_start(out=wk8_sb[:], in_=wk8_d.ap().rearrange("(c p) h -> p c h", p=P))
            x8t_sb = persist.tile([P, DC, JX], F8, tag="x8t", name="x8t_sb", bufs=2)
            x8t_r = x8t_d.ap().rearrange("(c p) j -> p c j", p=P)
            nc.gpsimd.dma_start(out=x8t_sb[:, :, 0:1024], in_=x8t_r[:, :, 0:1024])
            wq8_sb = small.tile([P, DC, H], F8, tag="wq8", name="wq8_sb", bufs=2)
            nc.gpsimd.dma_start(out=wq8_sb[:], in_=wq8_d.ap().rearrange("(c p) h -> p c h", p=P))
            x8rt_sb = persist.tile([P, DC, JX], F8, tag="x8rt", name="x8rt_sb", bufs=2)
            nc.gpsimd.dma_start(out=x8rt_sb[:], in_=x8rt_d.ap().rearrange("(c p) j -> p c j", p=P))
            if half < jm_pad:
                nc.gpsimd.dma_start(out=m8t_sb[:, :, half:jm_pad],
                                    in_=m8t_r[:, :, half:jm_pad])
            nc.gpsimd.dma_start(out=x8t_sb[:, :, 1024:2048], in_=x8t_r[:, :, 1024:2048])
            addm_sb = small.tile([P, MC], F32, tag="addm", name="addm_sb", bufs=2)
            nc.gpsimd.dma_start(out=addm_sb[:], in_=addm_d[:, :])
            m8_sb = persist.tile([P, MC, D], F8, tag="m8", name="m8_sb", bufs=2)
            nc.gpsimd.dma_start(out=m8_sb[:], in_=m8_d.ap().rearrange("(c p) d -> p c d", p=P))
            wga8_sb = small.tile([P, DC, E], F8, tag="wga8", name="wga8_sb", bufs=2)
            nc.gpsimd.dma_start(out=wga8_sb[:], in_=wga8_d.ap().rearrange("(c p) f -> p c f", p=P))
            wg8a_sb = small.tile([P, DC, E], F8, tag="wg8a", name="wg8a_sb", bufs=2)
            nc.gpsimd.dma_start(out=wg8a_sb[:], in_=wg8a_d.ap().rearrange("(c p) f -> p c f", p=P))
            wg8b_sb = small.tile([P, DC, E], F8, tag="wg8b", name="wg8b_sb", bufs=2)
            nc.gpsimd.dma_start(out=wg8b_sb[:], in_=wg8b_d.ap().rearrange("(c p) f -> p c f", p=P))
            # x in fp16 is RECONSTRUCTED on Pool from the two fp8 halves
            # (saves 1MB of HBM vs shipping it): x = x8a + x8b, ~0.13% err.
            # Pool is idle here and DVE must stay free for the R3 relu
            # chain that recycles the PSUM 's' buffers.
            xt_sb = persist.tile([P, DC, JX], F16, tag="xt", name="xt_sb")
            for c in range(DC):
                nc.gpsimd.tensor_tensor(xt_sb[:, c, :], x8t_sb[:, c, :],
                                        x8rt_sb[:, c, :], op=Alu.add)

            kT8 = persist.tile([P, HC, jm_pad], F8, tag="kT8", name="kT8")

            # Matmul PSUM writes must stay within one 2KB bank -> N<=512 f32.
            # stat_fn/mov_fn take (c, w): chunk index and width (2, or 1 for
            # the odd-nchunk tail, which runs as a plain contract-128 fp8
            # matmul — same real cycles/column as DoubleRow).
            def mm8_halves(ps, stat_fn, mov_fn, nchunk,
                           start=True, stop=True):
                last_c = ((nchunk - 1) // 2) * 2
                for c in range(0, nchunk, 2):
                    w = min(2, nchunk - c)
                    f = mm8 if w == 2 else mm
                    for h in range(BLK // 512):
                        f(ps[:, h * 512:(h + 1) * 512], stat_fn(c, w),
                          mov_fn(c, w, h), start and c == 0,
                          stop and c == last_c)

            # ---------------- unit emitters ----------------
            # kproj column blocks of up to 1024 (jm_pad is a mult. of 256)
            NKP = (jm_pad + BLK - 1) // BLK

            def kproj_unit(n, m, act_relu=False):
                j0 = n * BLK
                w = min(BLK, jm_pad - j0)
                psk = psb.tile([P, BLK], F32, tag="s", name="psk", bufs=3)
                runs = [(o, min(512, w - o)) for o in range(0, w, 512)]
                for c in range(0, DC, 2):
                    for o, ww in runs:
                        mm8(psk[:, o:o + ww],
                            wk8_sb[:, c:c + 2, m * P:(m + 1) * P],
                            m8t_sb[:, c:c + 2, j0 + o:j0 + o + ww],
                            c == 0, c == DC - 2)
                dst = kT8[:, m, j0:j0 + w]
                src = psk[:, 0:w]
                if act_relu:
                    # ACT is idle pre-exp, and Relu shares Exp's act table
                    nc.scalar.activation(dst, src, Act.Relu)
                else:
                    nc.vector.tensor_scalar_max(dst, src, 0.0)

            # two qT8 buffers: scores0 reads qT8[0] while qproj1 fills qT8[1]
            qT8s = {}

            def qproj_unit(b, m):
                if b not in qT8s:
                    qT8s[b] = small.tile([P, HC, BLK], F8, tag="qT8",
                                         name=f"qT8_{b}", bufs=2)
                psq = psb.tile([P, BLK], F32, tag="s", name="psq", bufs=3)
                jx0 = b * BLK
                mm8_halves(
                    psq,
                    lambda c, w: wq8_sb[:, c:c + w, m * P:(m + 1) * P],
                    lambda c, w, h: x8t_sb[:, c:c + w,
                                           jx0 + h * 512:jx0 + (h + 1) * 512],
                    DC)
                nc.vector.tensor_scalar_max(qT8s[b][:, m, :], psq[:], 0.0)

            p8s = [arena.tile([P, MC, BLK], F8, tag=f"p8_{b}", name=f"p8_{b}")
                   for b in range(NBLK)]

            def scores_unit(b, t):
                ps = psb.tile([P, BLK], F32, tag="s", name="ps_s", bufs=3)
                mm8_halves(
                    ps,
                    lambda c, w: kT8[:, c:c + w, t * P:(t + 1) * P],
                    lambda c, w, h: qT8s[b][:, c:c + w, h * 512:(h + 1) * 512],
                    HC)
                nc.scalar.activation(p8s[b][:, t, :], ps[:], Act.Exp,
                                     bias=addm_sb[:, t:t + 1], scale=SCALE)

            psLs = {}

            NCS = (MC + 1) // 2   # colsum steps (last may be single-chunk)

            def colsum_step(b, k):
                if b not in psLs:
                    psLs[b] = psb.tile([P, BLK], F32, tag="L", name=f"psL_{b}",
                                       bufs=1)
                c = 2 * k
                w = min(2, MC - c)
                f = mm8 if w == 2 else mm
                for h in range(BLK // 512):
                    f(psLs[b][:, h * 512:(h + 1) * 512], ones_8[:, 0:w, :],
                      p8s[b][:, c:c + w, h * 512:(h + 1) * 512],
                      c == 0, k == NCS - 1)

            recips = {}

            def recip_unit(b):
                recips[b] = small.tile([P, BLK], F32, tag="recipB",
                                       name=f"recipB_{b}", bufs=2)
                nc.vector.reciprocal(recips[b][:], psLs[b][:])

            attT = arena.tile([P, DC, BLK], F32, tag="attT", name="attT")
            attT8 = arena.tile([P, DC, BLK], F8, tag="attT8", name="attT8")

            def att_unit(b, m):
                psa = psb.tile([P, BLK], F32, tag="s", name="ps_a", bufs=3)
                mm8_halves(
                    psa,
                    lambda t, w: m8_sb[:, t:t + w, m * P:(m + 1) * P],
                    lambda t, w, h: p8s[b][:, t:t + w, h * 512:(h + 1) * 512],
                    MC)
                # GPSIMD cannot access PSUM, so DVE does the PSUM reads.
                # For the early chunks Pool casts attT->fp8 from SBUF (its
                # latency is hidden); for the late, gate-critical chunks
                # DVE writes the fp8 copy directly.
                if m < 2 and b == 0:
                    nc.vector.tensor_tensor(attT[:, m, :], psa[:],
                                            recips[b][:], op=Alu.mult)
                    nc.gpsimd.tensor_copy(attT8[:, m, :], attT[:, m, :])
                else:
                    nc.vector.tensor_tensor(attT8[:, m, :], psa[:],
                                            recips[b][:], op=Alu.mult)
                    nc.vector.tensor_tensor(attT[:, m, :], psa[:],
                                            recips[b][:], op=Alu.mult)

            outT = arena.tile([P, EC, BLK], F16, tag="outT", name="outT")

            def gate_unit(b, f):
                jx0 = b * BLK
                psg = psb.tile([P, BLK], F32, tag="s", name="psg", bufs=3)
                # x-half: error-feedback double-fp8 split (x8a@W8a +
                # x8b@W8a + x8a@W8b; the dropped x8b@W8b term is ~1.5e-3
                # rms in the logits) — 3 DoubleRow mms cost 0.75x of one
                # fp16 mm and reuse tensors already shipped for qproj/recon
                mm8_halves(
                    psg,
                    lambda c, w: wg8a_sb[:, c:c + w, f * P:(f + 1) * P],
                    lambda c, w, h: x8t_sb[:, c:c + w,
                                           jx0 + h * 512:jx0 + (h + 1) * 512],
                    DC, stop=False)
                mm8_halves(
                    psg,
                    lambda c, w: wg8a_sb[:, c:c + w, f * P:(f + 1) * P],
                    lambda c, w, h: x8rt_sb[:, c:c + w,
                                            jx0 + h * 512:jx0 + (h + 1) * 512],
                    DC, start=False, stop=False)
                mm8_halves(
                    psg,
                    lambda c, w: wg8b_sb[:, c:c + w, f * P:(f + 1) * P],
                    lambda c, w, h: x8t_sb[:, c:c + w,
                                           jx0 + h * 512:jx0 + (h + 1) * 512],
                    DC, start=False, stop=False)
                # att-half in fp8 DoubleRow
                mm8_halves(
                    psg,
                    lambda c, w: wga8_sb[:, c:c + w, f * P:(f + 1) * P],
                    lambda c, w, h: attT8[:, c:c + w, h * 512:(h + 1) * 512],
                    DC, start=False)
                gTf = small.tile([P, BLK], F32, tag="gTf", name="gTf", bufs=2)
                # tanh(logits/2): psg holds 32*logits (xt carries 0.5, wgx
                # carries 64)
                nc.scalar.activation(gTf[:], psg[:], Act.Tanh,
                                     scale=1.0 / (2.0 * WSCALE))
                res_f = (xt_sb[:, f, jx0:jx0 + BLK] if f < DC
                         else attT[:, f - DC, :])
                # out = (tanh + 1) * res', res' carries the 0.5
                nc.vector.scalar_tensor_tensor(outT[:, f, :], gTf[:], 1.0,
                                               res_f, op0=Alu.add,
                                               op1=Alu.mult)
                # output leaves TRANSPOSED ([E, JX]) in fp16; host undoes both
                nc.sync.dma_start(
                    out=out_d[f * P:(f + 1) * P, jx0:jx0 + BLK],
                    in_=outT[:, f, :])

            # ---------------- schedule ----------------
            # R0: kproj n=0 + qproj b=0, relus split ACT/DVE
            for m in range(HC):
                kproj_unit(0, m, act_relu=True)
                qproj_unit(0, m)
            # R3: scores0 (paced 3-deep against the ACT exp drain),
            # interleaved with every ready independent unit
            fillers = ([("kp", (n, m)) for n in range(1, NKP)
                        for m in range(HC)] +
                       [("qp", m) for m in range(HC)])
            cs_next = 0
            for t in range(MC):
                scores_unit(0, t)
                if fillers:
                    kind, arg = fillers.pop(0)
                    if kind == "kp":
                        kproj_unit(arg[0], arg[1])
                    else:
                        qproj_unit(1, arg)
                elif cs_next < NCS and t >= min(2 * cs_next + 2, MC):
                    colsum_step(0, cs_next)
                    cs_next += 1
            for kind, arg in fillers:
                if kind == "kp":
                    kproj_unit(arg[0], arg[1])
                else:
                    qproj_unit(1, arg)
            while cs_next < NCS:
                colsum_step(0, cs_next)
                cs_next += 1
            recip_unit(0)
            # R5: att0 interleaved with the first scores1 tiles
            n_r5 = min(MC, 6)
            scores_unit(1, 0)
            for m in range(DC):
                att_unit(0, m)
                if 1 + m < n_r5:
                    scores_unit(1, 1 + m)
            for t in range(DC + 1, n_r5):
                scores_unit(1, t)
            # R6: gate0 interleaved with the remaining scores1 + colsum1
            rest = [("s1", t) for t in range(n_r5, MC)]
            cs1_next = 0
            emitted1 = n_r5
            for f in range(EC):
                gate_unit(0, f)
                budget = 2
                while budget > 0 and (rest or cs1_next < NCS):
                    if rest:
                        _, t = rest.pop(0)
                        scores_unit(1, t)
                        emitted1 = t + 1
                        budget -= 1
                    elif cs1_next < NCS and min(2 * cs1_next + 2, MC) <= emitted1:
                        colsum_step(1, cs1_next)
                        cs1_next += 1
                        budget -= 1
                    else:
                        break
            while cs1_next < NCS:
                colsum_step(1, cs1_next)
                cs1_next += 1
            recip_unit(1)
            # R8
            for m in range(DC):
                att_unit(1, m)
            for f in range(EC):
                gate_unit(1, f)

        if hw_loop is not None:
            with tc.For_i(0, hw_loop, 1) as iv:
                body(iv)
        else:
            for _ in range(iters):
                body()

    nc.compile()
    return nc


_CACHE = {}


def _get_program(jm_pad):
    key = ("prog", jm_pad)
    if key not in _CACHE:
        _CACHE[key] = build_program_v2(jm_pad=jm_pad)
    return _CACHE[key]


def _make_in_maps(inputs, memory, mask, Wq, Wk, Wg):
    f8np = mybir.dt.np(F8)
    inputs = np.ascontiguousarray(inputs, dtype=np.float32)
    memory = np.ascontiguousarray(memory, dtype=np.float32)
    mask = np.asarray(mask)
    Wq = np.asarray(Wq, dtype=np.float32)
    Wk = np.asarray(Wk, dtype=np.float32)
    Wg = np.asarray(Wg, dtype=np.float32)
    # ---- mask compaction: keep only valid memory rows (their masked
    # counterparts contribute exactly 0 to att and L), pad to mult. of 128
    counts = mask.astype(np.int64).sum(axis=1)
    jm_pad = int(max(256, ((counts.max() + 255) // 256) * 256))
    _LAST_JM_PAD[0] = jm_pad
    MC = jm_pad // P
    mem_c = np.zeros((N_CORES, jm_pad, D), dtype=np.float32)
    addm = np.full((N_CORES, jm_pad), -1e30, dtype=np.float32)
    for b in range(N_CORES):
        idx = np.nonzero(mask[b])[0]
        mem_c[b, :len(idx)] = memory[b, idx]
        addm[b, :len(idx)] = -CEXP
    addm = np.ascontiguousarray(
        addm.reshape(N_CORES, MC, P).transpose(0, 2, 1))   # [B, P, MC]
    x8 = inputs.astype(f8np)
    x8r = (inputs - x8.astype(np.float32)).astype(f8np)
    m8 = np.ascontiguousarray(mem_c.astype(f8np))
    wq8 = np.ascontiguousarray(Wq.astype(f8np))
    wk8 = np.ascontiguousarray(Wk.astype(f8np))
    # the device returns OUT2 = 2*out (the tanh-form sigmoid's 0.5 is
    # applied on the host), so gate weights carry just WSCALE. The x-half
    # weights are an error-feedback fp8 pair (scaled so the residual
    # clears fp8's subnormal floor).
    wgx_s = WSCALE * Wg[:D]
    wg8a = wgx_s.astype(f8np)
    wg8b = np.ascontiguousarray((wgx_s - wg8a.astype(np.float32)).astype(f8np))
    wg8a = np.ascontiguousarray(wg8a)
    wga8 = np.ascontiguousarray((WSCALE * Wg[D:]).astype(f8np))
    return [
        {"x8t": np.ascontiguousarray(x8[b].T),
         "x8rt": np.ascontiguousarray(x8r[b].T),
         "m8": m8[b],
         "m8t": np.ascontiguousarray(m8[b].T),
         "addm": addm[b],
         "wq8": wq8, "wk8": wk8,
         "wg8a": wg8a, "wg8b": wg8b, "wga8": wga8}
        for b in range(N_CORES)
    ]


def kernel(inputs, memory, mask, Wq, Wk, Wg):
    in_maps = _make_in_maps(inputs, memory, mask, Wq, Wk, Wg)
    nc = _get_program(_LAST_JM_PAD[0])
    res = run_bass_kernel_spmd(nc, in_maps, core_ids=list(range(N_CORES)))
    # device ships OUT2 = 2*out transposed in fp16; undo both here
    return np.stack([0.5 * np.ascontiguousarray(res.results[b]["out"].T,
                                                dtype=np.float32).astype(np.float32)
                 for b in range(N_CORES)]).astype(np.float32)


# revision 19
# speedup vs baseline: 1.2193x; 1.2193x over previous
"""Trainium2 Bass kernel for nn_DotAttention (B=8, JX=JM=2048, D=H=512).

Sharding: data-parallel over batch B — one batch element per NeuronCore
(8 cores). Weights replicated. Per example:

    q  = relu(x @ Wq)          k = relu(mem @ Wk)
    s  = q @ k^T / sqrt(H)     p = exp(s + (mask-1)*1e30 - C)   (C=5: scores
                               are bounded ~[1.9, 8.8], so exp(s-C) <= ~50
                               fits fp8e4m3 and no row-max pass is needed)
    att = (p @ mem) / colsum(p)
    res = [x, att];  out = res * sigmoid(res @ Wg)

MASK COMPACTION: masked memory slots contribute EXACTLY zero (exp of
-1e30) to att and L, so the host gathers only the valid rows of
`memory` (per example, ~50% of JM) and pads to a multiple of 256.
kproj/scores/colsum/att then run on JM_pad ~ 1280 instead of 2048 —
identical math, ~37% less work in the attention path.

Precision plan (tolerance 2e-2 scale-relative; fp8 peak is 157 TF/s =
2x bf16, i.e. a 256-contract DoubleRow instruction costs the same
cycles/column as a 128-contract bf16 one):
  fp8e4m3 DoubleRow for kproj/qproj/scores/att and the gate GEMM's
  att-half; fp16 for the gate GEMM's x-half (x quantization error
  dominates the gate logits at fp8); fp16 output DMA (host upcasts).

The gate's sigmoid is computed as tanh: sigmoid(z) = 0.5*(1+tanh(z/2)),
because Tanh lives in the SAME activation-function table as Exp while
Sigmoid does not — interleaving exp and sigmoid on ACT would cost a
1283ns table reload per switch. The 0.5 factors are folded into
host-side xt = 0.5*x, colsum ones = 2.0 (so recip = 0.5/L and
attT = 0.5*att), and 2x-scaled gate weights; the +1 rides the fused
DVE scalar_tensor_tensor: out = (tanh + 1.0) * res'.

Schedule: PE work is interleaved so the ACT exp drain (which PSUM 's'
buffers rotate 3-deep against) always has independent matmuls to
overlap with:

  R0: kproj(n=0) + qproj(b=0) interleaved (k-relus on ACT, q on DVE)
  R3: scores0 all tiles, interleaved with kproj(n=1) | qproj(b=1) |
        early colsum0 steps
  R5: att0 interleaved with the first scores1 tiles
  R6: gate0 interleaved with the last scores1 tiles | colsum1
  R8: att1, gate1

DMA: ALL input triggers ride the Pool queue in prev-iteration
WAR-release order (Pool's sequencer wraps earliest, so inputs prefetch
during the previous iteration); outputs ride the SP queue.
All transposed operands are prepared on the HOST.
"""

import sys

for _p in ("/opt/trn_rl_repo",):
    if _p not in sys.path:
        sys.path.insert(0, _p)

import numpy as np

import concourse.bass as bass
import concourse.mybir as mybir
import concourse.tile as tile
from concourse import bacc
from concourse.bass_utils import run_bass_kernel_spmd
from contextlib import ExitStack

F32 = mybir.dt.float32
F16 = mybir.dt.float16
F8 = mybir.dt.float8e4

P = 128
JX = 2048
JM = 2048
D = 512
H = 512
E = 2 * D
N_CORES = 8
SCALE = 1.0 / float(np.sqrt(H))
CEXP = 5.0          # exp offset folded into the mask bias
WSCALE = 32.0       # gate weights are quantized at 32x; tanh rescales
BLK = 1024

Act = mybir.ActivationFunctionType
Alu = mybir.AluOpType
DR = mybir.MatmulPerfMode.DoubleRow

DC = D // P    # 4
HC = H // P    # 4
EC = E // P    # 8
NBLK = JX // BLK

# set by _make_in_maps (compacted memory length); 2048 = no compaction
_LAST_JM_PAD = [JM]


def build_program_v2(hw_loop=None, iters=1, enable_asserts=False, jm_pad=None):
    if jm_pad is None:
        jm_pad = _LAST_JM_PAD[0]
    MC = jm_pad // P
    nc = bacc.Bacc("TRN2", target_bir_lowering=False, debug=False,
                   enable_asserts=enable_asserts)

    x8t_d = nc.dram_tensor("x8t", [D, JX], F8, kind="ExternalInput")
    xt_d = nc.dram_tensor("xt", [D, JX], F16, kind="ExternalInput")
    m8_d = nc.dram_tensor("m8", [jm_pad, D], F8, kind="ExternalInput")
    m8t_d = nc.dram_tensor("m8t", [D, jm_pad], F8, kind="ExternalInput")
    addm_d = nc.dram_tensor("addm", [P, MC], F32, kind="ExternalInput")
    wq8_d = nc.dram_tensor("wq8", [D, H], F8, kind="ExternalInput")
    wk8_d = nc.dram_tensor("wk8", [D, H], F8, kind="ExternalInput")
    wgx_d = nc.dram_tensor("wgx", [D, E], F16, kind="ExternalInput")
    wga8_d = nc.dram_tensor("wga8", [D, E], F8, kind="ExternalInput")
    out_d = nc.dram_tensor("out", [E, JX], F16, kind="ExternalOutput")

    def mm(ps, lhsT, rhs, start, stop):
        nc.tensor.matmul(ps, lhsT, rhs, start=start, stop=stop)

    def mm8(ps, lhsT, rhs, start, stop):
        nc.tensor.matmul(ps, lhsT, rhs, start=start, stop=stop, perf_mode=DR)

    with tile.TileContext(nc) as tc, \
         nc.allow_low_precision(reason="fp8/fp16 mixed-precision plan, "
                                "validated vs 2e-2 tolerance"):
      with ExitStack() as ctx:
        const = ctx.enter_context(tc.tile_pool(name="const", bufs=1))
        # value 2.0: psL accumulates 2*L so recip lands at 0.5/L, folding
        # the tanh-form sigmoid's 0.5 into att (see module docstring)
        twos_f = const.tile([P, 2, P], F32)
        nc.vector.memset(twos_f[:], 2.0)
        twos_8 = const.tile([P, 2, P], F8)
        nc.scalar.copy(twos_8[:], twos_f[:])

        persist = ctx.enter_context(tc.tile_pool(name="persist", bufs=1))
        arena = ctx.enter_context(tc.tile_pool(name="arena", bufs=1))
        small = ctx.enter_context(tc.tile_pool(name="small", bufs=2))
        psb = ctx.enter_context(tc.tile_pool(name="psb", bufs=1, space="PSUM"))

        def body(_iv=None):
            # ---- input DMA triggers, ALL on the Pool queue, ordered by the
            # previous iteration's WAR-release time of each destination tile
            # (a blocked trigger stalls Pool.SEQ and every later trigger).
            m8t_sb = arena.tile([P, DC, jm_pad], F8, tag="m8t", name="m8t_sb")
            m8t_r = m8t_d.ap().rearrange("(c p) j -> p c j", p=P)
            half = min(1024, jm_pad)
            nc.gpsimd.dma_start(out=m8t_sb[:, :, 0:half], in_=m8t_r[:, :, 0:half])
            wk8_sb = small.tile([P, DC, H], F8, tag="wk8", name="wk8_sb", bufs=1)
            nc.gpsimd.dma_start(out=wk8_sb[:], in_=wk8_d.ap().rearrange("(c p) h -> p c h", p=P))
            x8t_sb = persist.tile([P, DC, JX], F8, tag="x8t", name="x8t_sb")
            x8t_r = x8t_d.ap().rearrange("(c p) j -> p c j", p=P)
            nc.gpsimd.dma_start(out=x8t_sb[:, :, 0:1024], in_=x8t_r[:, :, 0:1024])
            wq8_sb = small.tile([P, DC, H], F8, tag="wq8", name="wq8_sb", bufs=1)
            nc.gpsimd.dma_start(out=wq8_sb[:], in_=wq8_d.ap().rearrange("(c p) h -> p c h", p=P))
            if half < jm_pad:
                nc.gpsimd.dma_start(out=m8t_sb[:, :, half:jm_pad],
                                    in_=m8t_r[:, :, half:jm_pad])
            nc.gpsimd.dma_start(out=x8t_sb[:, :, 1024:2048], in_=x8t_r[:, :, 1024:2048])
            addm_sb = small.tile([P, MC], F32, tag="addm", name="addm_sb", bufs=1)
            nc.gpsimd.dma_start(out=addm_sb[:], in_=addm_d[:, :])
            m8_sb = persist.tile([P, MC, D], F8, tag="m8", name="m8_sb")
            nc.gpsimd.dma_start(out=m8_sb[:], in_=m8_d.ap().rearrange("(c p) d -> p c d", p=P))
            wga8_sb = small.tile([P, DC, E], F8, tag="wga8", name="wga8_sb", bufs=1)
            nc.gpsimd.dma_start(out=wga8_sb[:], in_=wga8_d.ap().rearrange("(c p) f -> p c f", p=P))
            wgx_sb = persist.tile([P, DC, E], F16, tag="wgx", name="wgx_sb")
            nc.gpsimd.dma_start(out=wgx_sb[:], in_=wgx_d.ap().rearrange("(c p) f -> p c f", p=P))
            xt_sb = persist.tile([P, DC, JX], F16, tag="xt", name="xt_sb")
            xt_r = xt_d.ap().rearrange("(c p) j -> p c j", p=P)
            for g in range(2):
                nc.gpsimd.dma_start(out=xt_sb[:, g * 2:(g + 1) * 2, :],
                                    in_=xt_r[:, g * 2:(g + 1) * 2, :])

            kT8 = persist.tile([P, HC, jm_pad], F8, tag="kT8", name="kT8")

            # Matmul PSUM writes must stay within one 2KB bank -> N<=512 f32.
            def mm8_halves(ps, stat_fn, mov_fn, nchunk, step=2,
                           start=True, stop=True):
                for c in range(0, nchunk, step):
                    for h in range(BLK // 512):
                        mm8(ps[:, h * 512:(h + 1) * 512], stat_fn(c),
                            mov_fn(c, h), start and c == 0,
                            stop and c == nchunk - step)

            # ---------------- unit emitters ----------------
            # kproj column blocks of up to 1024 (jm_pad is a mult. of 256)
            NKP = (jm_pad + BLK - 1) // BLK

            def kproj_unit(n, m, act_relu=False):
                j0 = n * BLK
                w = min(BLK, jm_pad - j0)
                psk = psb.tile([P, BLK], F32, tag="s", name="psk", bufs=3)
                runs = [(o, min(512, w - o)) for o in range(0, w, 512)]
                for c in range(0, DC, 2):
                    for o, ww in runs:
                        mm8(psk[:, o:o + ww],
                            wk8_sb[:, c:c + 2, m * P:(m + 1) * P],
                            m8t_sb[:, c:c + 2, j0 + o:j0 + o + ww],
                            c == 0, c == DC - 2)
                dst = kT8[:, m, j0:j0 + w]
                src = psk[:, 0:w]
                if act_relu:
                    # ACT is idle pre-exp, and Relu shares Exp's act table
                    nc.scalar.activation(dst, src, Act.Relu)
                else:
                    nc.vector.tensor_scalar_max(dst, src, 0.0)

            # two qT8 buffers: scores0 reads qT8[0] while qproj1 fills qT8[1]
            qT8s = {}

            def qproj_unit(b, m):
                if b not in qT8s:
                    qT8s[b] = small.tile([P, HC, BLK], F8, tag="qT8",
                                         name=f"qT8_{b}", bufs=2)
                psq = psb.tile([P, BLK], F32, tag="s", name="psq", bufs=3)
                jx0 = b * BLK
                mm8_halves(
                    psq,
                    lambda c: wq8_sb[:, c:c + 2, m * P:(m + 1) * P],
                    lambda c, h: x8t_sb[:, c:c + 2,
                                        jx0 + h * 512:jx0 + (h + 1) * 512],
                    DC)
                nc.vector.tensor_scalar_max(qT8s[b][:, m, :], psq[:], 0.0)

            p8s = [arena.tile([P, MC, BLK], F8, tag=f"p8_{b}", name=f"p8_{b}")
                   for b in range(NBLK)]

            def scores_unit(b, t):
                ps = psb.tile([P, BLK], F32, tag="s", name="ps_s", bufs=3)
                mm8_halves(
                    ps,
                    lambda c: kT8[:, c:c + 2, t * P:(t + 1) * P],
                    lambda c, h: qT8s[b][:, c:c + 2, h * 512:(h + 1) * 512],
                    HC)
                nc.scalar.activation(p8s[b][:, t, :], ps[:], Act.Exp,
                                     bias=addm_sb[:, t:t + 1], scale=SCALE)

            psLs = {}

            def colsum_step(b, k):
                if b not in psLs:
                    psLs[b] = psb.tile([P, BLK], F32, tag="L", name=f"psL_{b}",
                                       bufs=1)
                c = 2 * k
                for h in range(BLK // 512):
                    mm8(psLs[b][:, h * 512:(h + 1) * 512], twos_8[:],
                        p8s[b][:, c:c + 2, h * 512:(h + 1) * 512],
                        c == 0, c == MC - 2)

            recips = {}

            def recip_unit(b):
                recips[b] = small.tile([P, BLK], F32, tag="recipB",
                                       name=f"recipB_{b}", bufs=2)
                nc.vector.reciprocal(recips[b][:], psLs[b][:])

            attT = arena.tile([P, DC, BLK], F32, tag="attT", name="attT")
            attT8 = arena.tile([P, DC, BLK], F8, tag="attT8", name="attT8")

            def att_unit(b, m):
                psa = psb.tile([P, BLK], F32, tag="s", name="ps_a", bufs=3)
                mm8_halves(
                    psa,
                    lambda t: m8_sb[:, t:t + 2, m * P:(m + 1) * P],
                    lambda t, h: p8s[b][:, t:t + 2, h * 512:(h + 1) * 512],
                    MC)
                # GPSIMD cannot access PSUM, so DVE does the PSUM reads.
                # For the early chunks Pool casts attT->fp8 from SBUF (its
                # latency is hidden); for the late, gate-critical chunks
                # DVE writes the fp8 copy directly.
                if m < 2:
                    nc.vector.tensor_tensor(attT[:, m, :], psa[:],
                                            recips[b][:], op=Alu.mult)
                    nc.gpsimd.tensor_copy(attT8[:, m, :], attT[:, m, :])
                else:
                    nc.vector.tensor_tensor(attT8[:, m, :], psa[:],
                                            recips[b][:], op=Alu.mult)
                    nc.vector.tensor_tensor(attT[:, m, :], psa[:],
                                            recips[b][:], op=Alu.mult)

            outT = arena.tile([P, EC, BLK], F16, tag="outT", name="outT")

            def gate_unit(b, f):
                jx0 = b * BLK
                psg = psb.tile([P, BLK], F32, tag="s", name="psg", bufs=3)
                # x-half in fp16 (x's quantization error dominates at fp8)
                for e in range(DC):
                    for h in range(BLK // 512):
                        mm(psg[:, h * 512:(h + 1) * 512],
                           wgx_sb[:, e, f * P:(f + 1) * P],
                           xt_sb[:, e, jx0 + h * 512:jx0 + (h + 1) * 512],
                           e == 0, False)
                # att-half in fp8 DoubleRow
                mm8_halves(
                    psg,
                    lambda c: wga8_sb[:, c:c + 2, f * P:(f + 1) * P],
                    lambda c, h: attT8[:, c:c + 2, h * 512:(h + 1) * 512],
                    DC, start=False)
                gTf = small.tile([P, BLK], F32, tag="gTf", name="gTf", bufs=2)
                # tanh(logits/2): psg holds 32*logits (xt carries 0.5, wgx
                # carries 64)
                nc.scalar.activation(gTf[:], psg[:], Act.Tanh,
                                     scale=1.0 / (2.0 * WSCALE))
                res_f = (xt_sb[:, f, jx0:jx0 + BLK] if f < DC
                         else attT[:, f - DC, :])
                # out = (tanh + 1) * res', res' carries the 0.5
                nc.vector.scalar_tensor_tensor(outT[:, f, :], gTf[:], 1.0,
                                               res_f, op0=Alu.add,
                                               op1=Alu.mult)
                # output leaves TRANSPOSED ([E, JX]) in fp16; host undoes both
                nc.sync.dma_start(
                    out=out_d[f * P:(f + 1) * P, jx0:jx0 + BLK],
                    in_=outT[:, f, :])

            # ---------------- schedule ----------------
            # R0: kproj n=0 + qproj b=0, relus split ACT/DVE
            for m in range(HC):
                kproj_unit(0, m, act_relu=True)
                qproj_unit(0, m)
            # R3: scores0 (paced 3-deep against the ACT exp drain),
            # interleaved with every ready independent unit
            fillers = ([("kp", (n, m)) for n in range(1, NKP)
                        for m in range(HC)] +
                       [("qp", m) for m in range(HC)])
            cs_next = 0
            for t in range(MC):
                scores_unit(0, t)
                if fillers:
                    kind, arg = fillers.pop(0)
                    if kind == "kp":
                        kproj_unit(arg[0], arg[1])
                    else:
                        qproj_unit(1, arg)
                elif cs_next < MC // 2 and t >= 2 * cs_next + 2:
                    colsum_step(0, cs_next)
                    cs_next += 1
            for kind, arg in fillers:
                if kind == "kp":
                    kproj_unit(arg[0], arg[1])
                else:
                    qproj_unit(1, arg)
            while cs_next < MC // 2:
                colsum_step(0, cs_next)
                cs_next += 1
            recip_unit(0)
            # R5: att0 interleaved with the first scores1 tiles
            n_r5 = min(MC, 6)
            scores_unit(1, 0)
            for m in range(DC):
                att_unit(0, m)
                if 1 + m < n_r5:
                    scores_unit(1, 1 + m)
            for t in range(DC + 1, n_r5):
                scores_unit(1, t)
            # R6: gate0 interleaved with the remaining scores1 + colsum1
            rest = [("s1", t) for t in range(n_r5, MC)]
            cs1_next = 0
            emitted1 = n_r5
            for f in range(EC):
                gate_unit(0, f)
                budget = 2
                while budget > 0 and (rest or cs1_next < MC // 2):
                    if rest:
                        _, t = rest.pop(0)
                        scores_unit(1, t)
                        emitted1 = t + 1
                        budget -= 1
                    elif 2 * cs1_next + 2 <= emitted1:
                        colsum_step(1, cs1_next)
                        cs1_next += 1
                        budget -= 1
                    else:
                        break
            while cs1_next < MC // 2:
                colsum_step(1, cs1_next)
                cs1_next += 1
            recip_unit(1)
            # R8
            for m in range(DC):
                att_unit(1, m)
            for f in range(EC):
                gate_unit(1, f)

        if hw_loop is not None:
            with tc.For_i(0, hw_loop, 1) as iv:
                body(iv)
        else:
            for _ in range(iters):
                body()

    nc.compile()
    return nc


_CACHE = {}


def _get_program(jm_pad):
    key = ("prog", jm_pad)
    if key not in _CACHE:
        _CACHE[key] = build_program_v2(jm_pad=jm_pad)
    return _CACHE[key]


def _make_in_maps(inputs, memory, mask, Wq, Wk, Wg):
    f8np = mybir.dt.np(F8)
    inputs = np.ascontiguousarray(inputs, dtype=np.float32)
    memory = np.ascontiguousarray(memory, dtype=np.float32)
    mask = np.asarray(mask)
    Wq = np.asarray(Wq, dtype=np.float32)
    Wk = np.asarray(Wk, dtype=np.float32)
    Wg = np.asarray(Wg, dtype=np.float32)
    # ---- mask compaction: keep only valid memory rows (their masked
    # counterparts contribute exactly 0 to att and L), pad to mult. of 256
    counts = mask.astype(np.int64).sum(axis=1)
    jm_pad = int(max(256, ((counts.max() + 255) // 256) * 256))
    _LAST_JM_PAD[0] = jm_pad
    MC = jm_pad // P
    mem_c = np.zeros((N_CORES, jm_pad, D), dtype=np.float32)
    addm = np.full((N_CORES, jm_pad), -1e30, dtype=np.float32)
    for b in range(N_CORES):
        idx = np.nonzero(mask[b])[0]
        mem_c[b, :len(idx)] = memory[b, idx]
        addm[b, :len(idx)] = -CEXP
    addm = np.ascontiguousarray(
        addm.reshape(N_CORES, MC, P).transpose(0, 2, 1))   # [B, P, MC]
    x8 = inputs.astype(f8np)
    m8 = np.ascontiguousarray(mem_c.astype(f8np))
    wq8 = np.ascontiguousarray(Wq.astype(f8np))
    wk8 = np.ascontiguousarray(Wk.astype(f8np))
    # xt carries the folded 0.5; gate weights carry 2*WSCALE
    wgx = np.ascontiguousarray((2.0 * WSCALE * Wg[:D]).astype(np.float16))
    wga8 = np.ascontiguousarray((2.0 * WSCALE * Wg[D:]).astype(f8np))
    return [
        {"xt": np.ascontiguousarray((0.5 * inputs[b]).T.astype(np.float16)),
         "x8t": np.ascontiguousarray(x8[b].T),
         "m8": m8[b],
         "m8t": np.ascontiguousarray(m8[b].T),
         "addm": addm[b],
         "wq8": wq8, "wk8": wk8,
         "wgx": wgx, "wga8": wga8}
        for b in range(N_CORES)
    ]


def kernel(inputs, memory, mask, Wq, Wk, Wg):
    in_maps = _make_in_maps(inputs, memory, mask, Wq, Wk, Wg)
    nc = _get_program(_LAST_JM_PAD[0])
    res = run_bass_kernel_spmd(nc, in_maps, core_ids=list(range(N_CORES)))
    return np.stack([np.ascontiguousarray(res.results[b]["out"].T)
                 for b in range(N_CORES)]).astype(np.float32)
